# revision 19
# baseline (speedup 1.0000x reference)
"""GQA attention (B=2, S=2048, H=2048, 32 heads / 8 KV groups, rope, causal-masked
softmax, output projection) distributed over 8 Trainium2 NeuronCores.

Sharding: data parallel over batch (2) x tensor parallel over KV groups (4 group-pairs).
Core c handles batch c//4 and KV groups {2*(c%4), 2*(c%4)+1} (= 8 q heads). Each core
computes its partial output projection (attn_out_shard @ wo_cols_shard.T); the host
sums the 4 partials per batch (the "all-reduce") and adds bo.

v3.1 (vs v2): the whole per-qt finish chain (1/den, broadcast, normalize, outproj) is
deferred and paced into the NEXT qt block's attention groups so the PE never stalls on
it (the DVE reciprocal + pool cast run concurrently with the next block's attention);
the broadcast+normalize for both h of a head-pair share one PSUM alloc and one SEL
LDWEIGHTS (fewer sc-ring injections);
the last Q projection pass runs sc-major so each chunk's rope overlaps the next
chunk's matmuls (kills the phase-transition stall where qt0's mp=2 scores waited
~6us on QTrot[3]'s rope); psA allocates the "sc" tag before "av" so scores land on
PSUM banks whose phase-P eviction finishes first; input DMA is reordered with a
2-tile x prefetch buffer (x0-1, wkv, x2-15, cos/sin, wq, CST, wo) and x tiles DMA'd
in 512-col chunks, starting the KV pass at ~12us with no mid-phase DMA stalls (every
DMA stall resets the PE p-state ramp, a hidden ~40% matmul tax); causal-diagonal
score tiles are column-trimmed with the dm1 block packed adjacent (single exp over
[lo, 1024-lo)) saving PE and Act time.
"""
import sys

for _p in ("/opt/trn_rl_repo",):
    if _p not in sys.path:
        sys.path.append(_p)

import numpy as np
import ml_dtypes

S = 2048
H = 2048
HD = 64
NQT = 4          # s_q tiles of 512
NKT = 16         # s_k tiles of 128

_CACHE = {}


def _build(mode, has_bq, has_bk, has_bv):
    import concourse.bass as bass  # noqa: F401
    import concourse.mybir as mybir
    import concourse.tile as tile
    from concourse import bacc

    f32 = mybir.dt.float32
    b16 = mybir.dt.bfloat16
    AF = mybir.ActivationFunctionType
    ALU = mybir.AluOpType

    nc = bacc.Bacc("TRN2", target_bir_lowering=False, debug=False)
    xT = nc.dram_tensor("xT", [H, S], b16, kind="ExternalInput")
    wqT = nc.dram_tensor("wqT", [H, 512], b16, kind="ExternalInput")
    wkvT = nc.dram_tensor("wkvT", [H, 256], b16, kind="ExternalInput")
    woR = nc.dram_tensor("woR", [512, H], b16, kind="ExternalInput")
    COSd = nc.dram_tensor("COSx", [128, S], b16, kind="ExternalInput")
    SINd = nc.dram_tensor("SINx", [128, S], b16, kind="ExternalInput")
    # packed constants: [L (strict lower tri) | Z (-8e9*I) | I | SEL0..SEL3]
    CSTd = nc.dram_tensor("CST", [128, 640], b16, kind="ExternalInput")
    outd = nc.dram_tensor("out", [S, H], b16, kind="ExternalOutput")
    maskd = nc.dram_tensor("maskT", [S, S], f32, kind="ExternalInput") if mode == "generic" else None
    bqd = nc.dram_tensor("bq", [512, 1], f32, kind="ExternalInput") if has_bq else None
    bkvd = nc.dram_tensor("bkv", [256, 1], f32, kind="ExternalInput") if (has_bk or has_bv) else None

    with tile.TileContext(nc) as tc:
        with (
            tc.tile_pool(name="const", bufs=1) as cstp,
            tc.tile_pool(name="wts", bufs=1) as wts,
            tc.tile_pool(name="per", bufs=1) as per,
            tc.tile_pool(name="rtmp", bufs=2) as rtp,
            tc.tile_pool(name="et", bufs=8) as etp,
            tc.tile_pool(name="rcs", bufs=2) as rcp,
            tc.tile_pool(name="avqs", bufs=2) as avp,
            tc.tile_pool(name="outs", bufs=3) as outp,
            tc.tile_pool(name="mks", bufs=2) as mkp,
        ):
            # tiles first (alloc order is irrelevant for SBUF), DMAs ordered by
            # first use: biases, wkv+x chunks (KV pass), cos/sin (rope), CST
            # (V transpose + diag mask), wq (Q passes), wo (outproj)
            COS = cstp.tile([128, S], b16, tag="cos")
            SIN = cstp.tile([128, S], b16, tag="sin")
            CST = cstp.tile([128, 640], b16, tag="cst")
            Lc = CST[:, 0:128]       # L[k, m] = 1 if k < m
            Zc = CST[:, 128:256]     # -8e9 * I
            Ic = CST[:, 256:384]     # identity (transpose helper)
            # SEL[m]: all-ones row at partition 32m (recip broadcast lhsT)
            SELc = [CST[:, 384 + 64 * m:384 + 64 * (m + 1)] for m in range(4)]
            wkv_t = [wts.tile([128, 256], b16, tag=f"wkv{k}", name=f"wkv_t{k}") for k in range(16)]
            x_t = [wts.tile([128, S], b16, tag=f"xt{k}", name=f"x_t{k}") for k in range(16)]
            wq_t = [wts.tile([128, 512], b16, tag=f"wq{k}", name=f"wq_t{k}") for k in range(16)]
            wo_t = [wts.tile([128, S], b16, tag=f"wor{k}", name=f"wo_t{k}") for k in range(4)]

            if has_bq:
                bq_t = [cstp.tile([128, 1], f32, tag=f"bq{m}", name=f"bq_t{m}") for m in range(4)]
                for m in range(4):
                    nc.sync.dma_start(bq_t[m][:], bqd[128 * m:128 * (m + 1), :])
            if has_bk or has_bv:
                bk_t = cstp.tile([128, 1], f32, tag="bkt")
                bv_t = cstp.tile([128, 1], f32, tag="bvt")
                nc.sync.dma_start(bk_t[:], bkvd[0:128, :])
                nc.sync.dma_start(bv_t[:], bkvd[128:256, :])
            # x tiles in 512-col chunks (subtile deps: kv matmul (k, sc) waits only
            # for chunk sc of tile k). Prefetch x0-1 before wkv so once the KV
            # pass starts it never waits for DMA again (a DMA stall resets the
            # PE p-state ramp: ~40% slower matmuls for the next 3us).
            def dma_x(k):
                for c in range(4):
                    nc.sync.dma_start(x_t[k][:, 512 * c:512 * (c + 1)],
                                      xT[128 * k:128 * (k + 1), 512 * c:512 * (c + 1)])
            for k in range(4):
                dma_x(k)
            for k in range(16):
                nc.sync.dma_start(wkv_t[k][:], wkvT[128 * k:128 * (k + 1), :])
            for k in range(4, 16):
                dma_x(k)
            nc.sync.dma_start(COS[:], COSd[:])
            nc.sync.dma_start(SIN[:], SINd[:])
            for k in range(16):
                nc.sync.dma_start(wq_t[k][:], wqT[128 * k:128 * (k + 1), :])
            nc.sync.dma_start(CST[:], CSTd[:])
            for k in range(4):
                nc.sync.dma_start(wo_t[k][:], woR[128 * k:128 * (k + 1), :])

            # persistent intermediates (bf16 matmul operands)
            QTrot = [per.tile([128, S], b16, tag=f"qtrot{m}", name=f"QTrot{m}") for m in range(4)]
            KTrot = per.tile([128, S], b16, tag="ktrot")
            # V with a ones column per kt-block: [g0 v64 | 1 | g1 v64 | 1] x 16 kt
            Vp = per.tile([128, 130 * NKT], b16, tag="vp")
            nc.gpsimd.memset(Vp[:], 1.0)  # ones columns at 130*kt+{64,129} survive
            VTt = [per.tile([128, 512], b16, tag=f"vtt{sc}", name=f"VTt{sc}") for sc in range(4)]

            def rope(ps, dst_col_slice, dst):
                # evict psum -> bf16 (Act), then dst = COS*qs + SIN*(block-swapped qs)
                # on all-bf16 all-SBUF DVE ops (fast DVE modes)
                ssl = dst_col_slice
                qs = rtp.tile([128, 512], b16, tag="qs")
                nc.scalar.activation(qs[:], ps[:], AF.Copy)
                t1 = rtp.tile([128, 512], b16, tag="t1")
                t2 = rtp.tile([128, 512], b16, tag="t2")
                nc.vector.tensor_tensor(t1[:], qs[:], COS[:, ssl], ALU.mult)
                for blk in range(4):
                    sb = 32 * (blk ^ 1)
                    db = 32 * blk
                    # SIN is pre-swapped on host so both SBUF inputs share base sb
                    # (verifier: equal input base partitions when both are in SB)
                    nc.vector.tensor_tensor(
                        t2[db:db + 32, :], qs[sb:sb + 32, :], SIN[sb:sb + 32, ssl], ALU.mult)
                nc.vector.tensor_tensor(dst[:, ssl], t1[:], t2[:], ALU.add)

            # ---------- Phase P: projections + rope + V transpose ----------
            with tc.tile_pool(name="psP", bufs=1, space="PSUM") as psP:
                pp = lambda i: psP.tile([128, 512], f32, tag=f"pp{i}", name=f"pp{i}")

                # pass KV: kvK -> pp0-3, kvV -> pp4-7 (weight-stationary over 4 blocks)
                kvK = [pp(i) for i in range(4)]
                kvV = [pp(4 + i) for i in range(4)]
                for k in range(16):
                    st = (k == 0)
                    sp = (k == 15)
                    # last iteration does kvV first: its VTt evictions (which
                    # gate the V transposes and the m=0 Q pass banks) start
                    # while the final kvK matmuls still run
                    halves = ((0, 128), (128, 256)) if k < 15 else ((128, 256), (0, 128))
                    for c0, c1 in halves:
                        dst = kvK if c0 == 0 else kvV
                        for sc in range(4):
                            ssl = slice(512 * sc, 512 * (sc + 1))
                            nc.tensor.matmul(dst[sc][:], wkv_t[k][:, c0:c1], x_t[k][:, ssl], start=st, stop=sp)
                # VTt copies first so the PE's next work (the V transposes)
                # isn't queued behind the rope evictions on the Act engine
                for sc in range(4):
                    if has_bv:
                        nc.vector.tensor_scalar_add(kvV[sc][:], kvV[sc][:], bv_t[:])
                    nc.scalar.activation(VTt[sc][:], kvV[sc][:], AF.Copy)
                # V transpose: 16 kt blocks via PE transpose (bf16), scatter into Vp
                for kt in range(16):
                    sc, j = divmod(kt, 4)
                    vt = psP.tile([128, 512], f32, tag=f"pp{4 + (kt % 2)}", name=f"vt{kt}")
                    vps = vt[:].bitcast(b16)[:, 0:128]
                    nc.tensor.transpose(vps, VTt[sc][:, 128 * j:128 * (j + 1)], Ic)
                    dst = Vp[:, 130 * kt:130 * kt + 130].rearrange("p (two x) -> p two x", two=2)[:, :, 0:64]
                    src_ap = vps.rearrange("p (two x) -> p two x", two=2)
                    nc.vector.tensor_copy(dst, src_ap)
                for sc in range(4):
                    ssl = slice(512 * sc, 512 * (sc + 1))
                    if has_bk:
                        nc.vector.tensor_scalar_add(kvK[sc][:], kvK[sc][:], bk_t[:])
                    rope(kvK[sc], ssl, KTrot)

                # passes Q m=0..3: alternate pp0-3 / pp4-7. The LAST pass runs
                # sc-major (k inner) so each chunk's rope chain (Act evict + DVE
                # muls, ~3us) overlaps the next chunk's matmuls instead of
                # stalling qt0's mp=2 scores on QTrot[3].
                for m in range(4):
                    base = 4 if (m % 2 == 0) else 0
                    qp = [pp(base + i) for i in range(4)]
                    if m < 3:
                        for k in range(16):
                            st = (k == 0)
                            sp = (k == 15)
                            for sc in range(4):
                                ssl = slice(512 * sc, 512 * (sc + 1))
                                nc.tensor.matmul(qp[sc][:], wq_t[k][:, 128 * m:128 * (m + 1)],
                                                 x_t[k][:, ssl], start=st, stop=sp)
                        for sc in range(4):
                            ssl = slice(512 * sc, 512 * (sc + 1))
                            if has_bq:
                                nc.vector.tensor_scalar_add(qp[sc][:], qp[sc][:], bq_t[m][:])
                            rope(qp[sc], ssl, QTrot[m])
                    else:
                        for sc in (2, 3, 0, 1):
                            ssl = slice(512 * sc, 512 * (sc + 1))
                            for k in range(16):
                                nc.tensor.matmul(qp[sc][:], wq_t[k][:, 128 * m:128 * (m + 1)],
                                                 x_t[k][:, ssl], start=(k == 0), stop=(k == 15))
                            if has_bq:
                                nc.vector.tensor_scalar_add(qp[sc][:], qp[sc][:], bq_t[m][:])
                            rope(qp[sc], ssl, QTrot[m])

            # ---------- Phase A: attention; the whole finish chain of qt-1
            # (recip, broadcast, normalize, outproj) paced into qt ----------
            with tc.tile_pool(name="psA", bufs=1, space="PSUM") as psA:
                # allocate the "sc" tag first so it gets PSUM banks 0-3 (last
                # written by the m=2 Q pass, whose rope eviction completes during
                # the m=3 pass); "av" gets 4-7 (m=3's banks) but the first AV
                # matmul trails the phase boundary by ~3 groups, hiding m=3's
                # rope eviction.
                psA.tile([128, 1024], f32, tag="sc", bufs=3, name="sc_order")

                pending = []   # deferred finish units of the previous qt block

                def outproj_unit(avq_all, qt, j, npair):
                    def emit():
                        mm = 4 * qt + j
                        op = psA.tile([128, 1024], f32, tag="sc", bufs=3, name="op")
                        for k in range(4):
                            for nn in range(2):
                                nsl = slice(1024 * npair + 512 * nn, 1024 * npair + 512 * (nn + 1))
                                nc.tensor.matmul(op[:, 512 * nn:512 * (nn + 1)],
                                                 avq_all[k][:, 128 * j:128 * (j + 1)],
                                                 wo_t[k][:, nsl], start=(k == 0), stop=(k == 3))
                        ot = outp.tile([128, 1024], b16, tag="ot", name="ot")
                        nc.vector.tensor_copy(ot[:], op[:])
                        nc.sync.dma_start(
                            outd[128 * mm:128 * (mm + 1), 1024 * npair:1024 * (npair + 1)], ot[:])
                    return emit

                for qt in range(NQT):
                    qb0 = 512 * qt
                    causal = mode == "causal"
                    n_kt = 4 * qt + 4 if causal else NKT
                    avq_all = [avp.tile([128, 512], b16, tag=f"avq{m}", bufs=2, name=f"avq{m}")
                               for m in range(4)]
                    den = [rcp.tile([128, 512], f32, tag=f"den{h}", name=f"den{h}") for h in range(2)]
                    for h in range(2):
                        # unused rows must stay finite: recip of garbage could be
                        # inf/NaN and the broadcast matmul would poison the sums
                        nc.gpsimd.memset(den[h][:], 1.0)
                    # pace the previous block's finish units evenly through this
                    # block's attention groups so no engine ever stalls on them
                    # delay the paced units ~8 groups: the previous block's
                    # reciprocal chain (DVE recip x2 + pool cast) needs ~14us
                    # before the first broadcast can run without stalling
                    n_groups = 2 * n_kt
                    delay_g = min(8, max(n_groups - len(pending) - 2, 0))
                    pace_per_group = (len(pending) + 0.001) / max(n_groups - 2 - delay_g, 1)
                    pace_quota = -delay_g * pace_per_group

                    rbs = {}

                    def emit_recip(h, den=den, rbs=rbs, qt=qt):
                        if qt == NQT - 1 and h == 0:
                            # final drain: Act is idle once the last exp is done,
                            # so 1/den = Exp(-Ln(den)) there beats the ~5us DVE
                            # reciprocal + pool cast chain (2 table switches cost
                            # ~2.6us but nothing else wants the Act engine)
                            lt = rcp.tile([128, 512], f32, tag=f"rT{h}", name=f"rT{h}")
                            nc.scalar.activation(lt[:], den[h][:], AF.Ln)
                            rb = rcp.tile([128, 512], b16, tag=f"rb{h}", name=f"rb{h}")
                            nc.scalar.activation(rb[:], lt[:], AF.Exp, scale=-1.0)
                        else:
                            rT = rcp.tile([128, 512], f32, tag=f"rT{h}", name=f"rT{h}")
                            nc.vector.reciprocal(rT[:], den[h][:])
                            rb = rcp.tile([128, 512], b16, tag=f"rb{h}", name=f"rb{h}")
                            nc.gpsimd.tensor_copy(rb[:], rT[:])
                        rbs[h] = rb

                    # process q-tiles in m-PAIRS: scores/mask/AV matmuls for the two m
                    # share their stationary operand, so LDWEIGHTS amortizes over 2
                    # matmuls (the PE reloads weights serially between matmuls)
                    # hloc-major: "av" holds only the current (mp, hloc)
                    # sub-block's 2 accumulators (2 banks), freeing 2 banks to
                    # deepen the "sc" ring to 3 - the scores WAR on exp then has
                    # 1.5 groups of slack instead of 1, so the PE no longer eats
                    # a semaphore wait (and a p-state ramp reset) every group
                    for mp in (0, 2):
                        # mp=2 runs hloc=1 first: den[1] then completes a whole
                        # sub-block early, hiding its reciprocal chain
                        for hloc in ((0, 1) if mp == 0 else (1, 0)):
                            qb = 64 * hloc
                            av = {}
                            for dm in range(2):
                                av[dm] = psA.tile([128, 512], f32, tag="av", bufs=2,
                                                  name=f"av{dm}")
                            avdelay = []

                            def emit_av(job, av=av, hloc=hloc):
                                pr2, eTs = job
                                for half in range(2):
                                    kt = 2 * pr2 + half
                                    t = kt - 4 * qt
                                    lo = 128 * t if (causal and t >= 0) else 0
                                    vsl = Vp[:, 130 * kt + 65 * hloc:130 * kt + 65 * hloc + 65]
                                    for dm in range(2):
                                        # dm1 scores/exp are packed adjacent: eT cols
                                        # [512, 1024-lo) hold q positions [lo, 512)
                                        src = (eTs[half][:, lo:512] if dm == 0
                                               else eTs[half][:, 512:1024 - lo])
                                        nc.tensor.matmul(
                                            av[dm][0:65, lo:512], vsl, src,
                                            start=(kt == 0), stop=(kt == n_kt - 1))

                            for pr in range(n_kt // 2):
                                eTs = []
                                for half in range(2):
                                    kt = 2 * pr + half
                                    t = kt - 4 * qt
                                    # sc2 holds this kt's scores for BOTH m of the pair
                                    sc2 = psA.tile([128, 1024], f32, tag="sc", bufs=3, name="sc2")
                                    eT = etp.tile([128, 1024], b16, tag="eT", name="eT")
                                    ks = KTrot[64 * hloc:64 * hloc + 64, 128 * kt:128 * (kt + 1)]
                                    diag = (causal and t >= 0)
                                    lo = 128 * t if diag else 0
                                    for dm in range(2):
                                        osl = slice(lo, 512) if dm == 0 else slice(512, 1024 - lo)
                                        nc.tensor.matmul(
                                            sc2[:, osl], ks,
                                            QTrot[mp + dm][qb:qb + 64, qb0 + lo:qb0 + 512],
                                            start=True, stop=not diag)
                                    if diag:
                                        for dm in range(2):
                                            b0 = lo if dm == 0 else 512
                                            nc.tensor.matmul(sc2[:, b0:b0 + 128], Lc, Zc,
                                                             start=False, stop=True)
                                    if mode == "generic":
                                        mk = mkp.tile([128, 512], f32, tag="mk", name="mk")
                                        nc.sync.dma_start(mk[:], maskd[128 * kt:128 * (kt + 1), qb0:qb0 + 512])
                                        for dm in range(2):
                                            stt = mkp.tile([128, 512], f32, tag="stt", name="stt")
                                            nc.vector.scalar_tensor_tensor(
                                                stt[:], sc2[:, 512 * dm:512 * (dm + 1)], 0.125, mk[:],
                                                ALU.mult, ALU.add)
                                            nc.scalar.activation(
                                                eT[:, 512 * dm:512 * (dm + 1)], stt[:], AF.Exp, scale=1.0)
                                    else:
                                        nc.scalar.activation(eT[:, lo:1024 - lo],
                                                             sc2[:, lo:1024 - lo], AF.Exp, scale=0.125)
                                    eTs.append(eT)
                                # AV runs 3 groups behind its exp so the PE never
                                # waits on Act latency nor on the previous
                                # sub-block's avq evictions (av-ring WAR)
                                avdelay.append((pr, eTs))
                                if len(avdelay) > 3:
                                    emit_av(avdelay.pop(0))
                                pace_quota += pace_per_group
                                while pace_quota >= 1.0 and pending:
                                    pace_quota -= 1.0
                                    pending.pop(0)()
                            while avdelay:
                                emit_av(avdelay.pop(0))
                            # both den stagings before the avq evictions: the
                            # reciprocal chain starts ~0.7us sooner
                            for dm in range(2):
                                m = mp + dm
                                nc.vector.tensor_scalar_mul(den[hloc][32 * m:32 * m + 1, :],
                                                            av[dm][64:65, :], 1.0)
                            if mp == 2:
                                emit_recip(hloc)
                            for dm in range(2):
                                m = mp + dm
                                nc.vector.tensor_scalar_mul(avq_all[m][qb:qb + 64, :],
                                                            av[dm][0:64, :], 1.0)
                    while pending:   # safety: never drop undrained units
                        pending.pop(0)()

                    # reciprocals emitted inline (DVE + pool only - no PE cost);
                    # the PE-visible units (broadcast, outproj) are paced into
                    # the next block after a delay covering this chain

                    def bcast_norm_unit(m, avq_all=avq_all, rbs=rbs):
                        # both h of the pair share one PSUM alloc and the same SEL
                        # stationary (one LDWEIGHTS): fewer sc-ring injections
                        def emit():
                            rcb = psA.tile([128, 1024], f32, tag="sc", bufs=3, name="rcb")
                            for hloc in range(2):
                                nc.tensor.matmul(rcb[0:64, 512 * hloc:512 * (hloc + 1)],
                                                 SELc[m], rbs[hloc][:], start=True, stop=True)
                            for hloc in range(2):
                                qb = 64 * hloc
                                nc.vector.tensor_tensor(avq_all[m][qb:qb + 64, :],
                                                        avq_all[m][qb:qb + 64, :],
                                                        rcb[0:64, 512 * hloc:512 * (hloc + 1)],
                                                        ALU.mult)
                        return emit

                    pending = ([bcast_norm_unit(m) for m in range(4)]
                               + [outproj_unit(avq_all, qt, j, npair)
                                  for j in range(4) for npair in range(2)])
                for emit in pending:
                    emit()

    nc.compile()
    return nc


_PERM64 = np.concatenate([np.arange(0, 64, 2), np.arange(1, 64, 2)])
# Q-tile m holds local heads (m, m+4) so each head's partition base (0/64) matches
# its KV group's base in KTrot (group g at rows 64g) - matmul requires equal bases.
_HEADS_ORDER = np.array([0, 4, 1, 5, 2, 6, 3, 7])
_BF16 = ml_dtypes.bfloat16


def _prep_core(c, x, freqs_cis, mask, wq, bq, wk, bk, wv, bv, wo, mode,
               has_bq, has_bk, has_bv):
    b, gp = divmod(c, 4)
    f = np.float32
    xT = np.ascontiguousarray(x[b].T.astype(_BF16))
    wq_c = wq[512 * gp:512 * (gp + 1)].reshape(8, 64, H)[_HEADS_ORDER][:, _PERM64, :].reshape(512, H)
    wqT = np.ascontiguousarray(wq_c.T.astype(_BF16))
    wk_c = wk[128 * gp:128 * (gp + 1)].reshape(2, 64, H)[:, _PERM64, :].reshape(128, H)
    wv_c = wv[128 * gp:128 * (gp + 1)]
    wkvT = np.ascontiguousarray(np.concatenate([wk_c, wv_c], 0).T.astype(_BF16))
    woR = wo[:, 512 * gp:512 * (gp + 1)].T.reshape(8, 64, H)[_HEADS_ORDER].reshape(512, H)
    woR = np.ascontiguousarray(woR.astype(_BF16))
    cosT = np.ascontiguousarray(freqs_cis[:, 0::2].T, dtype=f)   # (32, S)
    sinT = np.ascontiguousarray(freqs_cis[:, 1::2].T, dtype=f)
    COS = np.tile(cosT, (4, 1)).astype(_BF16)
    # partition blocks pre-swapped pairwise: block at base sb holds the sign-applied
    # sin coefficients of destination block db = sb ^ 32 (see rope in _build)
    SIN = np.concatenate([sinT, -sinT, sinT, -sinT], 0).astype(_BF16)
    i = np.arange(128)
    L = (i[:, None] < i[None, :]).astype(_BF16)          # L[k, m] = 1 if k < m
    Z = (np.float32(-8e9) * np.eye(128, dtype=f)).astype(_BF16)
    I = np.eye(128, dtype=f).astype(_BF16)
    SEL = []
    for m4 in range(4):
        s = np.zeros((128, 64), dtype=_BF16)
        s[32 * m4, :] = 1
        SEL.append(s)
    CST = np.concatenate([L, Z, I] + SEL, axis=1)
    m = {"xT": xT, "wqT": wqT, "wkvT": wkvT, "woR": woR,
         "COSx": np.ascontiguousarray(COS), "SINx": np.ascontiguousarray(SIN),
         "CST": np.ascontiguousarray(CST)}
    if mode == "generic":
        m["maskT"] = np.ascontiguousarray(mask.T, dtype=f)
    if has_bq:
        bq_c = bq[512 * gp:512 * (gp + 1)].reshape(8, 64)[_HEADS_ORDER][:, _PERM64].reshape(512, 1)
        m["bq"] = np.ascontiguousarray(bq_c, dtype=f)
    if has_bk or has_bv:
        bk_c = bk[128 * gp:128 * (gp + 1)].reshape(2, 64)[:, _PERM64].reshape(128)
        bv_c = bv[128 * gp:128 * (gp + 1)]
        m["bkv"] = np.ascontiguousarray(np.concatenate([bk_c, bv_c]).reshape(256, 1), dtype=f)
    return m


def _detect_mode(mask):
    causal = np.where(np.tril(np.ones((S, S), dtype=bool)), np.float32(0.0), np.float32(-1e9))
    if np.array_equal(mask, causal):
        return "causal"
    if not np.any(mask):
        return "zeros"
    return "generic"


def _run(inputs, trace=False):
    from concourse import bass_utils
    x = np.asarray(inputs["x"], dtype=np.float32)
    freqs_cis = np.asarray(inputs["freqs_cis"], dtype=np.float32)
    mask = np.asarray(inputs["mask"], dtype=np.float32)
    wq = np.asarray(inputs["wq"], dtype=np.float32)
    bq = np.asarray(inputs["bq"], dtype=np.float32)
    wk = np.asarray(inputs["wk"], dtype=np.float32)
    bk = np.asarray(inputs["bk"], dtype=np.float32)
    wv = np.asarray(inputs["wv"], dtype=np.float32)
    bv = np.asarray(inputs["bv"], dtype=np.float32)
    wo = np.asarray(inputs["wo"], dtype=np.float32)
    bo = np.asarray(inputs["bo"], dtype=np.float32)

    mode = _detect_mode(mask)
    has_bq = bool(np.any(bq))
    has_bk = bool(np.any(bk))
    has_bv = bool(np.any(bv))
    key = (mode, has_bq, has_bk, has_bv)
    if key not in _CACHE:
        _CACHE[key] = _build(*key)
    nc = _CACHE[key]

    in_maps = [
        _prep_core(c, x, freqs_cis, mask, wq, bq, wk, bk, wv, bv, wo, mode,
                   has_bq, has_bk, has_bv)
        for c in range(8)
    ]
    res = bass_utils.run_bass_kernel_spmd(nc, in_maps, core_ids=list(range(8)), trace=trace)
    partials = np.stack([res.results[c]["out"].astype(np.float32) for c in range(8)], 0)
    out = partials.reshape(2, 4, S, H).sum(axis=1) + bo[None, None, :]
    return out.astype(np.float32), res


def kernel(**inputs):
    out, _ = _run(inputs, trace=False)
    return out


# revision 20
# speedup vs baseline: 1.0066x; 1.0066x over previous
"""GQA attention (B=2, S=2048, H=2048, 32 heads / 8 KV groups, rope, causal-masked
softmax, output projection) distributed over 8 Trainium2 NeuronCores.

Sharding: data parallel over batch (2) x tensor parallel over KV groups (4 group-pairs).
Core c handles batch c//4 and KV groups {2*(c%4), 2*(c%4)+1} (= 8 q heads). Each core
computes its partial output projection (attn_out_shard @ wo_cols_shard.T); the host
sums the 4 partials per batch (the "all-reduce") and adds bo.

v3.1 (vs v2): the whole per-qt finish chain (1/den, broadcast, normalize, outproj) is
deferred and paced into the NEXT qt block's attention groups so the PE never stalls on
it (the DVE reciprocal + pool cast run concurrently with the next block's attention);
the broadcast+normalize for both h of a head-pair share one PSUM alloc and one SEL
LDWEIGHTS (fewer sc-ring injections);
the last Q projection pass runs sc-major so each chunk's rope overlaps the next
chunk's matmuls (kills the phase-transition stall where qt0's mp=2 scores waited
~6us on QTrot[3]'s rope); psA allocates the "sc" tag before "av" so scores land on
PSUM banks whose phase-P eviction finishes first; input DMA is reordered with a
2-tile x prefetch buffer (x0-1, wkv, x2-15, cos/sin, wq, CST, wo) and x tiles DMA'd
in 512-col chunks, starting the KV pass at ~12us with no mid-phase DMA stalls (every
DMA stall resets the PE p-state ramp, a hidden ~40% matmul tax); causal-diagonal
score tiles are column-trimmed with the dm1 block packed adjacent (single exp over
[lo, 1024-lo)) saving PE and Act time.
"""
import sys

for _p in ("/opt/trn_rl_repo",):
    if _p not in sys.path:
        sys.path.append(_p)

import numpy as np
import ml_dtypes

S = 2048
H = 2048
HD = 64
NQT = 4          # s_q tiles of 512
NKT = 16         # s_k tiles of 128

_CACHE = {}


def _build(mode, has_bq, has_bk, has_bv):
    import concourse.bass as bass  # noqa: F401
    import concourse.mybir as mybir
    import concourse.tile as tile
    from concourse import bacc

    f32 = mybir.dt.float32
    b16 = mybir.dt.bfloat16
    AF = mybir.ActivationFunctionType
    ALU = mybir.AluOpType

    nc = bacc.Bacc("TRN2", target_bir_lowering=False, debug=False)
    xT = nc.dram_tensor("xT", [H, S], b16, kind="ExternalInput")
    wqT = nc.dram_tensor("wqT", [H, 512], b16, kind="ExternalInput")
    wkvT = nc.dram_tensor("wkvT", [H, 256], b16, kind="ExternalInput")
    woR = nc.dram_tensor("woR", [512, H], b16, kind="ExternalInput")
    COSd = nc.dram_tensor("COSx", [128, S], b16, kind="ExternalInput")
    SINd = nc.dram_tensor("SINx", [128, S], b16, kind="ExternalInput")
    # packed constants: [L (strict lower tri) | Z (-8e9*I) | I | SEL0..SEL3]
    CSTd = nc.dram_tensor("CST", [128, 640], b16, kind="ExternalInput")
    outd = nc.dram_tensor("out", [S, H], b16, kind="ExternalOutput")
    maskd = nc.dram_tensor("maskT", [S, S], f32, kind="ExternalInput") if mode == "generic" else None
    bqd = nc.dram_tensor("bq", [512, 1], f32, kind="ExternalInput") if has_bq else None
    bkvd = nc.dram_tensor("bkv", [256, 1], f32, kind="ExternalInput") if (has_bk or has_bv) else None

    with tile.TileContext(nc) as tc:
        with (
            tc.tile_pool(name="const", bufs=1) as cstp,
            tc.tile_pool(name="wts", bufs=1) as wts,
            tc.tile_pool(name="per", bufs=1) as per,
            tc.tile_pool(name="rtmp", bufs=2) as rtp,
            tc.tile_pool(name="et", bufs=8) as etp,
            tc.tile_pool(name="rcs", bufs=2) as rcp,
            tc.tile_pool(name="avqs", bufs=2) as avp,
            tc.tile_pool(name="outs", bufs=3) as outp,
            tc.tile_pool(name="mks", bufs=2) as mkp,
        ):
            # tiles first (alloc order is irrelevant for SBUF), DMAs ordered by
            # first use: biases, wkv+x chunks (KV pass), cos/sin (rope), CST
            # (V transpose + diag mask), wq (Q passes), wo (outproj)
            COS = cstp.tile([128, S], b16, tag="cos")
            SIN = cstp.tile([128, S], b16, tag="sin")
            CST = cstp.tile([128, 640], b16, tag="cst")
            Lc = CST[:, 0:128]       # L[k, m] = 1 if k < m
            Zc = CST[:, 128:256]     # -8e9 * I
            Ic = CST[:, 256:384]     # identity (transpose helper)
            # SEL[m]: all-ones row at partition 32m (recip broadcast lhsT)
            SELc = [CST[:, 384 + 64 * m:384 + 64 * (m + 1)] for m in range(4)]
            wkv_t = [wts.tile([128, 256], b16, tag=f"wkv{k}", name=f"wkv_t{k}") for k in range(16)]
            x_t = [wts.tile([128, S], b16, tag=f"xt{k}", name=f"x_t{k}") for k in range(16)]
            wq_t = [wts.tile([128, 512], b16, tag=f"wq{k}", name=f"wq_t{k}") for k in range(16)]
            wo_t = [wts.tile([128, S], b16, tag=f"wor{k}", name=f"wo_t{k}") for k in range(4)]

            if has_bq:
                bq_t = [cstp.tile([128, 1], f32, tag=f"bq{m}", name=f"bq_t{m}") for m in range(4)]
                for m in range(4):
                    nc.sync.dma_start(bq_t[m][:], bqd[128 * m:128 * (m + 1), :])
            if has_bk or has_bv:
                bk_t = cstp.tile([128, 1], f32, tag="bkt")
                bv_t = cstp.tile([128, 1], f32, tag="bvt")
                nc.sync.dma_start(bk_t[:], bkvd[0:128, :])
                nc.sync.dma_start(bv_t[:], bkvd[128:256, :])
            # x tiles in 512-col chunks (subtile deps: kv matmul (k, sc) waits only
            # for chunk sc of tile k). Prefetch x0-1 before wkv so once the KV
            # pass starts it never waits for DMA again (a DMA stall resets the
            # PE p-state ramp: ~40% slower matmuls for the next 3us).
            def dma_x(k):
                for c in range(4):
                    nc.sync.dma_start(x_t[k][:, 512 * c:512 * (c + 1)],
                                      xT[128 * k:128 * (k + 1), 512 * c:512 * (c + 1)])
            for k in range(4):
                dma_x(k)
            for k in range(16):
                nc.sync.dma_start(wkv_t[k][:], wkvT[128 * k:128 * (k + 1), :])
            for k in range(4, 16):
                dma_x(k)
            for k in range(16):
                nc.sync.dma_start(wq_t[k][:], wqT[128 * k:128 * (k + 1), :])
            nc.sync.dma_start(COS[:], COSd[:])
            nc.sync.dma_start(SIN[:], SINd[:])
            nc.sync.dma_start(CST[:], CSTd[:])
            for k in range(4):
                nc.sync.dma_start(wo_t[k][:], woR[128 * k:128 * (k + 1), :])

            # persistent intermediates (bf16 matmul operands)
            QTrot = [per.tile([128, S], b16, tag=f"qtrot{m}", name=f"QTrot{m}") for m in range(4)]
            KTrot = per.tile([128, S], b16, tag="ktrot")
            # V with a ones column per kt-block: [g0 v64 | 1 | g1 v64 | 1] x 16 kt
            Vp = per.tile([128, 130 * NKT], b16, tag="vp")
            nc.gpsimd.memset(Vp[:], 1.0)  # ones columns at 130*kt+{64,129} survive
            VTt = [per.tile([128, 512], b16, tag=f"vtt{sc}", name=f"VTt{sc}") for sc in range(4)]

            def rope(ps, dst_col_slice, dst):
                # evict psum -> bf16 (Act), then dst = COS*qs + SIN*(block-swapped qs)
                # on all-bf16 all-SBUF DVE ops (fast DVE modes)
                ssl = dst_col_slice
                qs = rtp.tile([128, 512], b16, tag="qs")
                nc.scalar.activation(qs[:], ps[:], AF.Copy)
                t1 = rtp.tile([128, 512], b16, tag="t1")
                t2 = rtp.tile([128, 512], b16, tag="t2")
                nc.vector.tensor_tensor(t1[:], qs[:], COS[:, ssl], ALU.mult)
                for blk in range(4):
                    sb = 32 * (blk ^ 1)
                    db = 32 * blk
                    # SIN is pre-swapped on host so both SBUF inputs share base sb
                    # (verifier: equal input base partitions when both are in SB)
                    nc.vector.tensor_tensor(
                        t2[db:db + 32, :], qs[sb:sb + 32, :], SIN[sb:sb + 32, ssl], ALU.mult)
                nc.vector.tensor_tensor(dst[:, ssl], t1[:], t2[:], ALU.add)

            # ---------- Phase P: projections + rope + V transpose ----------
            with tc.tile_pool(name="psP", bufs=1, space="PSUM") as psP:
                pp = lambda i: psP.tile([128, 512], f32, tag=f"pp{i}", name=f"pp{i}")

                # pass KV: kvK -> pp0-3, kvV -> pp4-7 (weight-stationary over 4 blocks)
                kvK = [pp(i) for i in range(4)]
                kvV = [pp(4 + i) for i in range(4)]
                for k in range(16):
                    st = (k == 0)
                    sp = (k == 15)
                    # last iteration does kvV first: its VTt evictions (which
                    # gate the V transposes and the m=0 Q pass banks) start
                    # while the final kvK matmuls still run
                    halves = ((0, 128), (128, 256)) if k < 15 else ((128, 256), (0, 128))
                    for c0, c1 in halves:
                        dst = kvK if c0 == 0 else kvV
                        for sc in range(4):
                            ssl = slice(512 * sc, 512 * (sc + 1))
                            nc.tensor.matmul(dst[sc][:], wkv_t[k][:, c0:c1], x_t[k][:, ssl], start=st, stop=sp)
                # VTt copies first so the PE's next work (the V transposes)
                # isn't queued behind the rope evictions on the Act engine
                for sc in range(4):
                    if has_bv:
                        nc.vector.tensor_scalar_add(kvV[sc][:], kvV[sc][:], bv_t[:])
                    nc.scalar.activation(VTt[sc][:], kvV[sc][:], AF.Copy)
                # V transpose: 16 kt blocks via PE transpose (bf16), scatter into Vp
                for kt in range(16):
                    sc, j = divmod(kt, 4)
                    vt = psP.tile([128, 512], f32, tag=f"pp{4 + (kt % 2)}", name=f"vt{kt}")
                    vps = vt[:].bitcast(b16)[:, 0:128]
                    nc.tensor.transpose(vps, VTt[sc][:, 128 * j:128 * (j + 1)], Ic)
                    dst = Vp[:, 130 * kt:130 * kt + 130].rearrange("p (two x) -> p two x", two=2)[:, :, 0:64]
                    src_ap = vps.rearrange("p (two x) -> p two x", two=2)
                    nc.vector.tensor_copy(dst, src_ap)
                for sc in range(4):
                    ssl = slice(512 * sc, 512 * (sc + 1))
                    if has_bk:
                        nc.vector.tensor_scalar_add(kvK[sc][:], kvK[sc][:], bk_t[:])
                    rope(kvK[sc], ssl, KTrot)

                # passes Q m=0..3: alternate pp0-3 / pp4-7. The LAST pass runs
                # sc-major (k inner) so each chunk's rope chain (Act evict + DVE
                # muls, ~3us) overlaps the next chunk's matmuls instead of
                # stalling qt0's mp=2 scores on QTrot[3].
                for m in range(4):
                    base = 4 if (m % 2 == 0) else 0
                    qp = [pp(base + i) for i in range(4)]
                    if m < 3:
                        for k in range(16):
                            st = (k == 0)
                            sp = (k == 15)
                            for sc in range(4):
                                ssl = slice(512 * sc, 512 * (sc + 1))
                                nc.tensor.matmul(qp[sc][:], wq_t[k][:, 128 * m:128 * (m + 1)],
                                                 x_t[k][:, ssl], start=st, stop=sp)
                        for sc in range(4):
                            ssl = slice(512 * sc, 512 * (sc + 1))
                            if has_bq:
                                nc.vector.tensor_scalar_add(qp[sc][:], qp[sc][:], bq_t[m][:])
                            rope(qp[sc], ssl, QTrot[m])
                    else:
                        for sc in (2, 3, 0, 1):
                            ssl = slice(512 * sc, 512 * (sc + 1))
                            for k in range(16):
                                nc.tensor.matmul(qp[sc][:], wq_t[k][:, 128 * m:128 * (m + 1)],
                                                 x_t[k][:, ssl], start=(k == 0), stop=(k == 15))
                            if has_bq:
                                nc.vector.tensor_scalar_add(qp[sc][:], qp[sc][:], bq_t[m][:])
                            rope(qp[sc], ssl, QTrot[m])

            # ---------- Phase A: attention; the whole finish chain of qt-1
            # (recip, broadcast, normalize, outproj) paced into qt ----------
            with tc.tile_pool(name="psA", bufs=1, space="PSUM") as psA:
                # allocate the "sc" tag first so it gets PSUM banks 0-3 (last
                # written by the m=2 Q pass, whose rope eviction completes during
                # the m=3 pass); "av" gets 4-7 (m=3's banks) but the first AV
                # matmul trails the phase boundary by ~3 groups, hiding m=3's
                # rope eviction.
                psA.tile([128, 1024], f32, tag="sc", bufs=3, name="sc_order")

                pending = []   # deferred finish units of the previous qt block

                def outproj_unit(avq_all, qt, j, npair):
                    def emit():
                        mm = 4 * qt + j
                        op = psA.tile([128, 1024], f32, tag="sc", bufs=3, name="op")
                        for k in range(4):
                            for nn in range(2):
                                nsl = slice(1024 * npair + 512 * nn, 1024 * npair + 512 * (nn + 1))
                                nc.tensor.matmul(op[:, 512 * nn:512 * (nn + 1)],
                                                 avq_all[k][:, 128 * j:128 * (j + 1)],
                                                 wo_t[k][:, nsl], start=(k == 0), stop=(k == 3))
                        ot = outp.tile([128, 1024], b16, tag="ot", name="ot")
                        nc.vector.tensor_copy(ot[:], op[:])
                        nc.sync.dma_start(
                            outd[128 * mm:128 * (mm + 1), 1024 * npair:1024 * (npair + 1)], ot[:])
                    return emit

                for qt in range(NQT):
                    qb0 = 512 * qt
                    causal = mode == "causal"
                    n_kt = 4 * qt + 4 if causal else NKT
                    avq_all = [avp.tile([128, 512], b16, tag=f"avq{m}", bufs=2, name=f"avq{m}")
                               for m in range(4)]
                    den = [rcp.tile([128, 512], f32, tag=f"den{h}", name=f"den{h}") for h in range(2)]
                    for h in range(2):
                        # unused rows must stay finite: recip of garbage could be
                        # inf/NaN and the broadcast matmul would poison the sums
                        nc.gpsimd.memset(den[h][:], 1.0)
                    # pace the previous block's finish units evenly through this
                    # block's attention groups so no engine ever stalls on them
                    # delay the paced units ~8 groups: the previous block's
                    # reciprocal chain (DVE recip x2 + pool cast) needs ~14us
                    # before the first broadcast can run without stalling
                    n_groups = 2 * n_kt
                    delay_g = min(8, max(n_groups - len(pending) - 2, 0))
                    pace_per_group = (len(pending) + 0.001) / max(n_groups - 2 - delay_g, 1)
                    pace_quota = -delay_g * pace_per_group

                    rbs = {}

                    def emit_recip(h, den=den, rbs=rbs):
                        rT = rcp.tile([128, 512], f32, tag=f"rT{h}", name=f"rT{h}")
                        nc.vector.reciprocal(rT[:], den[h][:])
                        rb = rcp.tile([128, 512], b16, tag=f"rb{h}", name=f"rb{h}")
                        nc.gpsimd.tensor_copy(rb[:], rT[:])
                        rbs[h] = rb

                    # process q-tiles in m-PAIRS: scores/mask/AV matmuls for the two m
                    # share their stationary operand, so LDWEIGHTS amortizes over 2
                    # matmuls (the PE reloads weights serially between matmuls)
                    # hloc-major: "av" holds only the current (mp, hloc)
                    # sub-block's 2 accumulators (2 banks), freeing 2 banks to
                    # deepen the "sc" ring to 3 - the scores WAR on exp then has
                    # 1.5 groups of slack instead of 1, so the PE no longer eats
                    # a semaphore wait (and a p-state ramp reset) every group
                    for mp in (0, 2):
                        # mp=2 runs hloc=1 first: den[1] then completes a whole
                        # sub-block early, hiding its reciprocal chain
                        for hloc in ((0, 1) if mp == 0 else (1, 0)):
                            qb = 64 * hloc
                            av = {}
                            for dm in range(2):
                                av[dm] = psA.tile([128, 512], f32, tag="av", bufs=2,
                                                  name=f"av{dm}")
                            avdelay = []

                            def emit_av(job, av=av, hloc=hloc):
                                pr2, eTs = job
                                for half in range(2):
                                    kt = 2 * pr2 + half
                                    t = kt - 4 * qt
                                    lo = 128 * t if (causal and t >= 0) else 0
                                    vsl = Vp[:, 130 * kt + 65 * hloc:130 * kt + 65 * hloc + 65]
                                    for dm in range(2):
                                        # dm1 scores/exp are packed adjacent: eT cols
                                        # [512, 1024-lo) hold q positions [lo, 512)
                                        src = (eTs[half][:, lo:512] if dm == 0
                                               else eTs[half][:, 512:1024 - lo])
                                        nc.tensor.matmul(
                                            av[dm][0:65, lo:512], vsl, src,
                                            start=(kt == 0), stop=(kt == n_kt - 1))

                            for pr in range(n_kt // 2):
                                eTs = []
                                for half in range(2):
                                    kt = 2 * pr + half
                                    t = kt - 4 * qt
                                    # sc2 holds this kt's scores for BOTH m of the pair
                                    sc2 = psA.tile([128, 1024], f32, tag="sc", bufs=3, name="sc2")
                                    eT = etp.tile([128, 1024], b16, tag="eT", name="eT")
                                    ks = KTrot[64 * hloc:64 * hloc + 64, 128 * kt:128 * (kt + 1)]
                                    diag = (causal and t >= 0)
                                    lo = 128 * t if diag else 0
                                    for dm in range(2):
                                        osl = slice(lo, 512) if dm == 0 else slice(512, 1024 - lo)
                                        nc.tensor.matmul(
                                            sc2[:, osl], ks,
                                            QTrot[mp + dm][qb:qb + 64, qb0 + lo:qb0 + 512],
                                            start=True, stop=not diag)
                                    if diag:
                                        for dm in range(2):
                                            b0 = lo if dm == 0 else 512
                                            nc.tensor.matmul(sc2[:, b0:b0 + 128], Lc, Zc,
                                                             start=False, stop=True)
                                    if mode == "generic":
                                        mk = mkp.tile([128, 512], f32, tag="mk", name="mk")
                                        nc.sync.dma_start(mk[:], maskd[128 * kt:128 * (kt + 1), qb0:qb0 + 512])
                                        for dm in range(2):
                                            stt = mkp.tile([128, 512], f32, tag="stt", name="stt")
                                            nc.vector.scalar_tensor_tensor(
                                                stt[:], sc2[:, 512 * dm:512 * (dm + 1)], 0.125, mk[:],
                                                ALU.mult, ALU.add)
                                            nc.scalar.activation(
                                                eT[:, 512 * dm:512 * (dm + 1)], stt[:], AF.Exp, scale=1.0)
                                    else:
                                        nc.scalar.activation(eT[:, lo:1024 - lo],
                                                             sc2[:, lo:1024 - lo], AF.Exp, scale=0.125)
                                    eTs.append(eT)
                                # AV runs 3 groups behind its exp so the PE never
                                # waits on Act latency nor on the previous
                                # sub-block's avq evictions (av-ring WAR)
                                avdelay.append((pr, eTs))
                                if len(avdelay) > 3:
                                    emit_av(avdelay.pop(0))
                                pace_quota += pace_per_group
                                while pace_quota >= 1.0 and pending:
                                    pace_quota -= 1.0
                                    pending.pop(0)()
                            while avdelay:
                                emit_av(avdelay.pop(0))
                            last_sub = (mp == 2 and hloc == 0 and qt == NQT - 1)
                            if last_sub:
                                # nothing runs after this sub-block: put both den
                                # stagings and the reciprocal ahead of the avq
                                # evictions so the drain's chain starts sooner
                                for dm in range(2):
                                    m = mp + dm
                                    nc.vector.tensor_scalar_mul(den[hloc][32 * m:32 * m + 1, :],
                                                                av[dm][64:65, :], 1.0)
                                emit_recip(hloc)
                                for dm in range(2):
                                    m = mp + dm
                                    nc.vector.tensor_scalar_mul(avq_all[m][qb:qb + 64, :],
                                                                av[dm][0:64, :], 1.0)
                            else:
                                for dm in range(2):
                                    m = mp + dm
                                    # stage denominator row at partition 32m; evict unnormalized AV
                                    nc.vector.tensor_scalar_mul(den[hloc][32 * m:32 * m + 1, :],
                                                                av[dm][64:65, :], 1.0)
                                    nc.vector.tensor_scalar_mul(avq_all[m][qb:qb + 64, :],
                                                                av[dm][0:64, :], 1.0)
                                if mp == 2:
                                    emit_recip(hloc)
                    while pending:   # safety: never drop undrained units
                        pending.pop(0)()

                    # reciprocals emitted inline (DVE + pool only - no PE cost);
                    # the PE-visible units (broadcast, outproj) are paced into
                    # the next block after a delay covering this chain

                    def bcast_norm_unit(m, avq_all=avq_all, rbs=rbs):
                        # both h of the pair share one PSUM alloc and the same SEL
                        # stationary (one LDWEIGHTS): fewer sc-ring injections
                        def emit():
                            rcb = psA.tile([128, 1024], f32, tag="sc", bufs=3, name="rcb")
                            for hloc in range(2):
                                nc.tensor.matmul(rcb[0:64, 512 * hloc:512 * (hloc + 1)],
                                                 SELc[m], rbs[hloc][:], start=True, stop=True)
                            for hloc in range(2):
                                qb = 64 * hloc
                                nc.vector.tensor_tensor(avq_all[m][qb:qb + 64, :],
                                                        avq_all[m][qb:qb + 64, :],
                                                        rcb[0:64, 512 * hloc:512 * (hloc + 1)],
                                                        ALU.mult)
                        return emit

                    pending = ([bcast_norm_unit(m) for m in range(4)]
                               + [outproj_unit(avq_all, qt, j, npair)
                                  for j in range(4) for npair in range(2)])
                for emit in pending:
                    emit()

    nc.compile()
    return nc


_PERM64 = np.concatenate([np.arange(0, 64, 2), np.arange(1, 64, 2)])
# Q-tile m holds local heads (m, m+4) so each head's partition base (0/64) matches
# its KV group's base in KTrot (group g at rows 64g) - matmul requires equal bases.
_HEADS_ORDER = np.array([0, 4, 1, 5, 2, 6, 3, 7])
_BF16 = ml_dtypes.bfloat16


def _prep_core(c, x, freqs_cis, mask, wq, bq, wk, bk, wv, bv, wo, mode,
               has_bq, has_bk, has_bv):
    b, gp = divmod(c, 4)
    f = np.float32
    xT = np.ascontiguousarray(x[b].T.astype(_BF16))
    wq_c = wq[512 * gp:512 * (gp + 1)].reshape(8, 64, H)[_HEADS_ORDER][:, _PERM64, :].reshape(512, H)
    wqT = np.ascontiguousarray(wq_c.T.astype(_BF16))
    wk_c = wk[128 * gp:128 * (gp + 1)].reshape(2, 64, H)[:, _PERM64, :].reshape(128, H)
    wv_c = wv[128 * gp:128 * (gp + 1)]
    wkvT = np.ascontiguousarray(np.concatenate([wk_c, wv_c], 0).T.astype(_BF16))
    woR = wo[:, 512 * gp:512 * (gp + 1)].T.reshape(8, 64, H)[_HEADS_ORDER].reshape(512, H)
    woR = np.ascontiguousarray(woR.astype(_BF16))
    cosT = np.ascontiguousarray(freqs_cis[:, 0::2].T, dtype=f)   # (32, S)
    sinT = np.ascontiguousarray(freqs_cis[:, 1::2].T, dtype=f)
    COS = np.tile(cosT, (4, 1)).astype(_BF16)
    # partition blocks pre-swapped pairwise: block at base sb holds the sign-applied
    # sin coefficients of destination block db = sb ^ 32 (see rope in _build)
    SIN = np.concatenate([sinT, -sinT, sinT, -sinT], 0).astype(_BF16)
    i = np.arange(128)
    L = (i[:, None] < i[None, :]).astype(_BF16)          # L[k, m] = 1 if k < m
    Z = (np.float32(-8e9) * np.eye(128, dtype=f)).astype(_BF16)
    I = np.eye(128, dtype=f).astype(_BF16)
    SEL = []
    for m4 in range(4):
        s = np.zeros((128, 64), dtype=_BF16)
        s[32 * m4, :] = 1
        SEL.append(s)
    CST = np.concatenate([L, Z, I] + SEL, axis=1)
    m = {"xT": xT, "wqT": wqT, "wkvT": wkvT, "woR": woR,
         "COSx": np.ascontiguousarray(COS), "SINx": np.ascontiguousarray(SIN),
         "CST": np.ascontiguousarray(CST)}
    if mode == "generic":
        m["maskT"] = np.ascontiguousarray(mask.T, dtype=f)
    if has_bq:
        bq_c = bq[512 * gp:512 * (gp + 1)].reshape(8, 64)[_HEADS_ORDER][:, _PERM64].reshape(512, 1)
        m["bq"] = np.ascontiguousarray(bq_c, dtype=f)
    if has_bk or has_bv:
        bk_c = bk[128 * gp:128 * (gp + 1)].reshape(2, 64)[:, _PERM64].reshape(128)
        bv_c = bv[128 * gp:128 * (gp + 1)]
        m["bkv"] = np.ascontiguousarray(np.concatenate([bk_c, bv_c]).reshape(256, 1), dtype=f)
    return m


def _detect_mode(mask):
    causal = np.where(np.tril(np.ones((S, S), dtype=bool)), np.float32(0.0), np.float32(-1e9))
    if np.array_equal(mask, causal):
        return "causal"
    if not np.any(mask):
        return "zeros"
    return "generic"


def _run(inputs, trace=False):
    from concourse import bass_utils
    x = np.asarray(inputs["x"], dtype=np.float32)
    freqs_cis = np.asarray(inputs["freqs_cis"], dtype=np.float32)
    mask = np.asarray(inputs["mask"], dtype=np.float32)
    wq = np.asarray(inputs["wq"], dtype=np.float32)
    bq = np.asarray(inputs["bq"], dtype=np.float32)
    wk = np.asarray(inputs["wk"], dtype=np.float32)
    bk = np.asarray(inputs["bk"], dtype=np.float32)
    wv = np.asarray(inputs["wv"], dtype=np.float32)
    bv = np.asarray(inputs["bv"], dtype=np.float32)
    wo = np.asarray(inputs["wo"], dtype=np.float32)
    bo = np.asarray(inputs["bo"], dtype=np.float32)

    mode = _detect_mode(mask)
    has_bq = bool(np.any(bq))
    has_bk = bool(np.any(bk))
    has_bv = bool(np.any(bv))
    key = (mode, has_bq, has_bk, has_bv)
    if key not in _CACHE:
        _CACHE[key] = _build(*key)
    nc = _CACHE[key]

    in_maps = [
        _prep_core(c, x, freqs_cis, mask, wq, bq, wk, bk, wv, bv, wo, mode,
                   has_bq, has_bk, has_bv)
        for c in range(8)
    ]
    res = bass_utils.run_bass_kernel_spmd(nc, in_maps, core_ids=list(range(8)), trace=trace)
    partials = np.stack([res.results[c]["out"].astype(np.float32) for c in range(8)], 0)
    out = partials.reshape(2, 4, S, H).sum(axis=1) + bo[None, None, :]
    return out.astype(np.float32), res


def kernel(**inputs):
    out, _ = _run(inputs, trace=False)
    return out


# revision 21
# speedup vs baseline: 1.0524x; 1.0456x over previous
"""GQA attention (B=2, S=2048, H=2048, 32 heads / 8 KV groups, rope, causal-masked
softmax, output projection) distributed over 8 Trainium2 NeuronCores.

Sharding: data parallel over batch (2) x tensor parallel over KV groups (4 group-pairs).
Core c handles batch c//4 and KV groups {2*(c%4), 2*(c%4)+1} (= 8 q heads). Each core
computes its partial output projection (attn_out_shard @ wo_cols_shard.T); the host
sums the 4 partials per batch (the "all-reduce") and adds bo.

v3.1 (vs v2): the whole per-qt finish chain (1/den, broadcast, normalize, outproj) is
deferred and paced into the NEXT qt block's attention groups so the PE never stalls on
it (the DVE reciprocal + pool cast run concurrently with the next block's attention);
the broadcast+normalize for both h of a head-pair share one PSUM alloc and one SEL
LDWEIGHTS (fewer sc-ring injections);
the last Q projection pass runs sc-major so each chunk's rope overlaps the next
chunk's matmuls (kills the phase-transition stall where qt0's mp=2 scores waited
~6us on QTrot[3]'s rope); psA allocates the "sc" tag before "av" so scores land on
PSUM banks whose phase-P eviction finishes first; input DMA is reordered with a
2-tile x prefetch buffer (x0-1, wkv, x2-15, cos/sin, wq, CST, wo) and x tiles DMA'd
in 512-col chunks, starting the KV pass at ~12us with no mid-phase DMA stalls (every
DMA stall resets the PE p-state ramp, a hidden ~40% matmul tax); causal-diagonal
score tiles are column-trimmed with the dm1 block packed adjacent (single exp over
[lo, 1024-lo)) saving PE and Act time.
"""
import sys

for _p in ("/opt/trn_rl_repo",):
    if _p not in sys.path:
        sys.path.append(_p)

import numpy as np
import ml_dtypes

S = 2048
H = 2048
HD = 64
NQT = 4          # s_q tiles of 512
NKT = 16         # s_k tiles of 128

_CACHE = {}


def _build(mode, has_bq, has_bk, has_bv):
    import concourse.bass as bass  # noqa: F401
    import concourse.mybir as mybir
    import concourse.tile as tile
    from concourse import bacc

    f32 = mybir.dt.float32
    b16 = mybir.dt.bfloat16
    AF = mybir.ActivationFunctionType
    ALU = mybir.AluOpType

    nc = bacc.Bacc("TRN2", target_bir_lowering=False, debug=False)
    xT = nc.dram_tensor("xT", [H, S], b16, kind="ExternalInput")
    wqT = nc.dram_tensor("wqT", [H, 512], b16, kind="ExternalInput")
    wkvT = nc.dram_tensor("wkvT", [H, 256], b16, kind="ExternalInput")
    woR = nc.dram_tensor("woR", [512, H], b16, kind="ExternalInput")
    COSd = nc.dram_tensor("COSx", [128, S], b16, kind="ExternalInput")
    SINd = nc.dram_tensor("SINx", [128, S], b16, kind="ExternalInput")
    # packed constants: [L (strict lower tri) | Z (-8e9*I) | I | SEL0..SEL3]
    CSTd = nc.dram_tensor("CST", [128, 640], b16, kind="ExternalInput")
    outd = nc.dram_tensor("out", [S, H], b16, kind="ExternalOutput")
    maskd = nc.dram_tensor("maskT", [S, S], f32, kind="ExternalInput") if mode == "generic" else None
    bqd = nc.dram_tensor("bq", [512, 1], f32, kind="ExternalInput") if has_bq else None
    bkvd = nc.dram_tensor("bkv", [256, 1], f32, kind="ExternalInput") if (has_bk or has_bv) else None

    with tile.TileContext(nc) as tc:
        with (
            tc.tile_pool(name="const", bufs=1) as cstp,
            tc.tile_pool(name="wts", bufs=1) as wts,
            tc.tile_pool(name="per", bufs=1) as per,
            tc.tile_pool(name="rtmp", bufs=2) as rtp,
            tc.tile_pool(name="et", bufs=8) as etp,
            tc.tile_pool(name="rcs", bufs=2) as rcp,
            tc.tile_pool(name="avqs", bufs=2) as avp,
            tc.tile_pool(name="outs", bufs=3) as outp,
            tc.tile_pool(name="mks", bufs=2) as mkp,
        ):
            # tiles first (alloc order is irrelevant for SBUF), DMAs ordered by
            # first use: biases, wkv+x chunks (KV pass), cos/sin (rope), CST
            # (V transpose + diag mask), wq (Q passes), wo (outproj)
            COS = cstp.tile([128, S], b16, tag="cos")
            SIN = cstp.tile([128, S], b16, tag="sin")
            CST = cstp.tile([128, 640], b16, tag="cst")
            Lc = CST[:, 0:128]       # L[k, m] = 1 if k < m
            Zc = CST[:, 128:256]     # -8e9 * I
            Ic = CST[:, 256:384]     # identity (transpose helper)
            # SEL[m]: all-ones row at partition 32m (recip broadcast lhsT)
            SELc = [CST[:, 384 + 64 * m:384 + 64 * (m + 1)] for m in range(4)]
            wkv_t = [wts.tile([128, 256], b16, tag=f"wkv{k}", name=f"wkv_t{k}") for k in range(16)]
            x_t = [wts.tile([128, S], b16, tag=f"xt{k}", name=f"x_t{k}") for k in range(16)]
            wq_t = [wts.tile([128, 512], b16, tag=f"wq{k}", name=f"wq_t{k}") for k in range(16)]
            wo_t = [wts.tile([128, S], b16, tag=f"wor{k}", name=f"wo_t{k}") for k in range(4)]

            if has_bq:
                bq_t = [cstp.tile([128, 1], f32, tag=f"bq{m}", name=f"bq_t{m}") for m in range(4)]
                for m in range(4):
                    nc.sync.dma_start(bq_t[m][:], bqd[128 * m:128 * (m + 1), :])
            if has_bk or has_bv:
                bk_t = cstp.tile([128, 1], f32, tag="bkt")
                bv_t = cstp.tile([128, 1], f32, tag="bvt")
                nc.sync.dma_start(bk_t[:], bkvd[0:128, :])
                nc.sync.dma_start(bv_t[:], bkvd[128:256, :])
            # x tiles in 512-col chunks (subtile deps: kv matmul (k, sc) waits only
            # for chunk sc of tile k). Prefetch x0-1 before wkv so once the KV
            # pass starts it never waits for DMA again (a DMA stall resets the
            # PE p-state ramp: ~40% slower matmuls for the next 3us).
            def dma_x(k):
                for c in range(4):
                    nc.sync.dma_start(x_t[k][:, 512 * c:512 * (c + 1)],
                                      xT[128 * k:128 * (k + 1), 512 * c:512 * (c + 1)])
            for k in range(4):
                dma_x(k)
            for k in range(16):
                nc.sync.dma_start(wkv_t[k][:], wkvT[128 * k:128 * (k + 1), :])
            for k in range(4, 16):
                dma_x(k)
            for k in range(16):
                nc.sync.dma_start(wq_t[k][:], wqT[128 * k:128 * (k + 1), :])
            nc.sync.dma_start(COS[:], COSd[:])
            nc.sync.dma_start(SIN[:], SINd[:])
            nc.sync.dma_start(CST[:], CSTd[:])
            for k in range(4):
                nc.sync.dma_start(wo_t[k][:], woR[128 * k:128 * (k + 1), :])

            # persistent intermediates (bf16 matmul operands)
            QTrot = [per.tile([128, S], b16, tag=f"qtrot{m}", name=f"QTrot{m}") for m in range(4)]
            KTrot = per.tile([128, S], b16, tag="ktrot")
            # V with a ones column per kt-block: [g0 v64 | 1 | g1 v64 | 1] x 16 kt
            Vp = per.tile([128, 130 * NKT], b16, tag="vp")
            nc.gpsimd.memset(Vp[:], 1.0)  # ones columns at 130*kt+{64,129} survive
            VTt = [per.tile([128, 512], b16, tag=f"vtt{sc}", name=f"VTt{sc}") for sc in range(4)]

            def rope(ps, dst_col_slice, dst):
                # evict psum -> bf16 (Act), then dst = COS*qs + SIN*(block-swapped qs)
                # on all-bf16 all-SBUF DVE ops (fast DVE modes)
                ssl = dst_col_slice
                qs = rtp.tile([128, 512], b16, tag="qs")
                nc.scalar.activation(qs[:], ps[:], AF.Copy)
                t1 = rtp.tile([128, 512], b16, tag="t1")
                t2 = rtp.tile([128, 512], b16, tag="t2")
                nc.vector.tensor_tensor(t1[:], qs[:], COS[:, ssl], ALU.mult)
                for blk in range(4):
                    sb = 32 * (blk ^ 1)
                    db = 32 * blk
                    # SIN is pre-swapped on host so both SBUF inputs share base sb
                    # (verifier: equal input base partitions when both are in SB)
                    nc.vector.tensor_tensor(
                        t2[db:db + 32, :], qs[sb:sb + 32, :], SIN[sb:sb + 32, ssl], ALU.mult)
                nc.vector.tensor_tensor(dst[:, ssl], t1[:], t2[:], ALU.add)

            # ---------- Phase P: projections + rope + V transpose ----------
            with tc.tile_pool(name="psP", bufs=1, space="PSUM") as psP:
                pp = lambda i: psP.tile([128, 512], f32, tag=f"pp{i}", name=f"pp{i}")

                # pass KV: kvK -> pp0-3, kvV -> pp4-7 (weight-stationary over 4 blocks)
                kvK = [pp(i) for i in range(4)]
                kvV = [pp(4 + i) for i in range(4)]
                for k in range(16):
                    st = (k == 0)
                    sp = (k == 15)
                    # last iteration does kvV first: its VTt evictions (which
                    # gate the V transposes and the m=0 Q pass banks) start
                    # while the final kvK matmuls still run
                    halves = ((0, 128), (128, 256)) if k < 15 else ((128, 256), (0, 128))
                    for c0, c1 in halves:
                        dst = kvK if c0 == 0 else kvV
                        for sc in range(4):
                            ssl = slice(512 * sc, 512 * (sc + 1))
                            nc.tensor.matmul(dst[sc][:], wkv_t[k][:, c0:c1], x_t[k][:, ssl], start=st, stop=sp)
                # VTt copies first so the PE's next work (the V transposes)
                # isn't queued behind the rope evictions on the Act engine
                for sc in range(4):
                    if has_bv:
                        nc.vector.tensor_scalar_add(kvV[sc][:], kvV[sc][:], bv_t[:])
                    nc.scalar.activation(VTt[sc][:], kvV[sc][:], AF.Copy)
                # V transpose: 16 kt blocks via PE transpose (bf16), scatter into Vp
                for kt in range(16):
                    sc, j = divmod(kt, 4)
                    vt = psP.tile([128, 512], f32, tag=f"pp{4 + (kt % 2)}", name=f"vt{kt}")
                    vps = vt[:].bitcast(b16)[:, 0:128]
                    nc.tensor.transpose(vps, VTt[sc][:, 128 * j:128 * (j + 1)], Ic)
                    dst = Vp[:, 130 * kt:130 * kt + 130].rearrange("p (two x) -> p two x", two=2)[:, :, 0:64]
                    src_ap = vps.rearrange("p (two x) -> p two x", two=2)
                    nc.vector.tensor_copy(dst, src_ap)
                for sc in range(4):
                    ssl = slice(512 * sc, 512 * (sc + 1))
                    if has_bk:
                        nc.vector.tensor_scalar_add(kvK[sc][:], kvK[sc][:], bk_t[:])
                    rope(kvK[sc], ssl, KTrot)

                # passes Q m=0..3: alternate pp0-3 / pp4-7. The LAST pass runs
                # sc-major (k inner) so each chunk's rope chain (Act evict + DVE
                # muls, ~3us) overlaps the next chunk's matmuls instead of
                # stalling qt0's mp=2 scores on QTrot[3].
                for m in range(4):
                    base = 4 if (m % 2 == 0) else 0
                    qp = [pp(base + i) for i in range(4)]
                    if m < 3:
                        for k in range(16):
                            st = (k == 0)
                            sp = (k == 15)
                            for sc in range(4):
                                ssl = slice(512 * sc, 512 * (sc + 1))
                                nc.tensor.matmul(qp[sc][:], wq_t[k][:, 128 * m:128 * (m + 1)],
                                                 x_t[k][:, ssl], start=st, stop=sp)
                        for sc in range(4):
                            ssl = slice(512 * sc, 512 * (sc + 1))
                            if has_bq:
                                nc.vector.tensor_scalar_add(qp[sc][:], qp[sc][:], bq_t[m][:])
                            rope(qp[sc], ssl, QTrot[m])
                    else:
                        for sc in (2, 3, 0, 1):
                            ssl = slice(512 * sc, 512 * (sc + 1))
                            for k in range(16):
                                nc.tensor.matmul(qp[sc][:], wq_t[k][:, 128 * m:128 * (m + 1)],
                                                 x_t[k][:, ssl], start=(k == 0), stop=(k == 15))
                            if has_bq:
                                nc.vector.tensor_scalar_add(qp[sc][:], qp[sc][:], bq_t[m][:])
                            rope(qp[sc], ssl, QTrot[m])

            # ---------- Phase A: attention; the whole finish chain of qt-1
            # (recip, broadcast, normalize, outproj) paced into qt ----------
            with tc.tile_pool(name="psA", bufs=1, space="PSUM") as psA:
                # allocate the "sc" tag first so it gets PSUM banks 0-3 (last
                # written by the m=2 Q pass, whose rope eviction completes during
                # the m=3 pass); "av" gets 4-7 (m=3's banks) but the first AV
                # matmul trails the phase boundary by ~3 groups, hiding m=3's
                # rope eviction.
                psA.tile([128, 1024], f32, tag="sc", bufs=3, name="sc_order")

                pending = []   # deferred finish units of the previous qt block

                def outproj_unit(avq_all, qt, j, npair):
                    def emit():
                        mm = 4 * qt + j
                        op = psA.tile([128, 1024], f32, tag="sc", bufs=3, name="op")
                        for k in range(4):
                            for nn in range(2):
                                nsl = slice(1024 * npair + 512 * nn, 1024 * npair + 512 * (nn + 1))
                                nc.tensor.matmul(op[:, 512 * nn:512 * (nn + 1)],
                                                 avq_all[k][:, 128 * j:128 * (j + 1)],
                                                 wo_t[k][:, nsl], start=(k == 0), stop=(k == 3))
                        ot = outp.tile([128, 1024], b16, tag="ot", name="ot")
                        nc.vector.tensor_copy(ot[:], op[:])
                        nc.sync.dma_start(
                            outd[128 * mm:128 * (mm + 1), 1024 * npair:1024 * (npair + 1)], ot[:])
                    return emit

                for qt in range(NQT):
                    qb0 = 512 * qt
                    causal = mode == "causal"
                    n_kt = 4 * qt + 4 if causal else NKT
                    avq_all = [avp.tile([128, 512], b16, tag=f"avq{m}", bufs=2, name=f"avq{m}")
                               for m in range(4)]
                    den = [rcp.tile([128, 512], f32, tag=f"den{h}", name=f"den{h}") for h in range(2)]
                    for h in range(2):
                        # unused rows must stay finite: recip of garbage could be
                        # inf/NaN and the broadcast matmul would poison the sums
                        nc.gpsimd.memset(den[h][:], 1.0)
                    # pace the previous block's finish units evenly through this
                    # block's attention groups so no engine ever stalls on them
                    # delay the paced units ~8 groups: the previous block's
                    # reciprocal chain (DVE recip x2 + pool cast) needs ~14us
                    # before the first broadcast can run without stalling
                    n_groups = 2 * n_kt
                    delay_g = min(8, max(n_groups - len(pending) - 2, 0))
                    pace_per_group = (len(pending) + 0.001) / max(n_groups - 2 - delay_g, 1)
                    pace_quota = -delay_g * pace_per_group

                    rbs = {}

                    def emit_recip(h, den=den, rbs=rbs):
                        # bf16 output directly: same rounding the old pool cast
                        # applied, but no second hop in the latency chain
                        rb = rcp.tile([128, 512], b16, tag=f"rb{h}", name=f"rb{h}")
                        with nc.allow_low_precision("recip feeds a bf16 matmul operand"):
                            nc.vector.reciprocal(rb[:], den[h][:])
                        rbs[h] = rb

                    # process q-tiles in m-PAIRS: scores/mask/AV matmuls for the two m
                    # share their stationary operand, so LDWEIGHTS amortizes over 2
                    # matmuls (the PE reloads weights serially between matmuls)
                    # hloc-major: "av" holds only the current (mp, hloc)
                    # sub-block's 2 accumulators (2 banks), freeing 2 banks to
                    # deepen the "sc" ring to 3 - the scores WAR on exp then has
                    # 1.5 groups of slack instead of 1, so the PE no longer eats
                    # a semaphore wait (and a p-state ramp reset) every group
                    for mp in (0, 2):
                        # mp=2 runs hloc=1 first: den[1] then completes a whole
                        # sub-block early, hiding its reciprocal chain
                        for hloc in ((0, 1) if mp == 0 else (1, 0)):
                            qb = 64 * hloc
                            av = {}
                            for dm in range(2):
                                av[dm] = psA.tile([128, 512], f32, tag="av", bufs=2,
                                                  name=f"av{dm}")
                            avdelay = []

                            def emit_av(job, av=av, hloc=hloc):
                                pr2, eTs = job
                                for half in range(2):
                                    kt = 2 * pr2 + half
                                    t = kt - 4 * qt
                                    lo = 128 * t if (causal and t >= 0) else 0
                                    vsl = Vp[:, 130 * kt + 65 * hloc:130 * kt + 65 * hloc + 65]
                                    for dm in range(2):
                                        # dm1 scores/exp are packed adjacent: eT cols
                                        # [512, 1024-lo) hold q positions [lo, 512)
                                        src = (eTs[half][:, lo:512] if dm == 0
                                               else eTs[half][:, 512:1024 - lo])
                                        nc.tensor.matmul(
                                            av[dm][0:65, lo:512], vsl, src,
                                            start=(kt == 0), stop=(kt == n_kt - 1))

                            for pr in range(n_kt // 2):
                                eTs = []
                                for half in range(2):
                                    kt = 2 * pr + half
                                    t = kt - 4 * qt
                                    # sc2 holds this kt's scores for BOTH m of the pair
                                    sc2 = psA.tile([128, 1024], f32, tag="sc", bufs=3, name="sc2")
                                    eT = etp.tile([128, 1024], b16, tag="eT", name="eT")
                                    ks = KTrot[64 * hloc:64 * hloc + 64, 128 * kt:128 * (kt + 1)]
                                    diag = (causal and t >= 0)
                                    lo = 128 * t if diag else 0
                                    for dm in range(2):
                                        osl = slice(lo, 512) if dm == 0 else slice(512, 1024 - lo)
                                        nc.tensor.matmul(
                                            sc2[:, osl], ks,
                                            QTrot[mp + dm][qb:qb + 64, qb0 + lo:qb0 + 512],
                                            start=True, stop=not diag)
                                    if diag:
                                        for dm in range(2):
                                            b0 = lo if dm == 0 else 512
                                            nc.tensor.matmul(sc2[:, b0:b0 + 128], Lc, Zc,
                                                             start=False, stop=True)
                                    if mode == "generic":
                                        mk = mkp.tile([128, 512], f32, tag="mk", name="mk")
                                        nc.sync.dma_start(mk[:], maskd[128 * kt:128 * (kt + 1), qb0:qb0 + 512])
                                        for dm in range(2):
                                            stt = mkp.tile([128, 512], f32, tag="stt", name="stt")
                                            nc.vector.scalar_tensor_tensor(
                                                stt[:], sc2[:, 512 * dm:512 * (dm + 1)], 0.125, mk[:],
                                                ALU.mult, ALU.add)
                                            nc.scalar.activation(
                                                eT[:, 512 * dm:512 * (dm + 1)], stt[:], AF.Exp, scale=1.0)
                                    else:
                                        nc.scalar.activation(eT[:, lo:1024 - lo],
                                                             sc2[:, lo:1024 - lo], AF.Exp, scale=0.125)
                                    eTs.append(eT)
                                # AV runs 3 groups behind its exp so the PE never
                                # waits on Act latency nor on the previous
                                # sub-block's avq evictions (av-ring WAR)
                                avdelay.append((pr, eTs))
                                if len(avdelay) > 3:
                                    emit_av(avdelay.pop(0))
                                pace_quota += pace_per_group
                                while pace_quota >= 1.0 and pending:
                                    pace_quota -= 1.0
                                    pending.pop(0)()
                            while avdelay:
                                emit_av(avdelay.pop(0))
                            last_sub = (mp == 2 and hloc == 0 and qt == NQT - 1)
                            if last_sub:
                                # nothing runs after this sub-block: put both den
                                # stagings and the reciprocal ahead of the avq
                                # evictions so the drain's chain starts sooner
                                for dm in range(2):
                                    m = mp + dm
                                    nc.vector.tensor_scalar_mul(den[hloc][32 * m:32 * m + 1, :],
                                                                av[dm][64:65, :], 1.0)
                                emit_recip(hloc)
                                for dm in range(2):
                                    m = mp + dm
                                    nc.vector.tensor_scalar_mul(avq_all[m][qb:qb + 64, :],
                                                                av[dm][0:64, :], 1.0)
                            else:
                                for dm in range(2):
                                    m = mp + dm
                                    # stage denominator row at partition 32m; evict unnormalized AV
                                    nc.vector.tensor_scalar_mul(den[hloc][32 * m:32 * m + 1, :],
                                                                av[dm][64:65, :], 1.0)
                                    nc.vector.tensor_scalar_mul(avq_all[m][qb:qb + 64, :],
                                                                av[dm][0:64, :], 1.0)
                                if mp == 2:
                                    emit_recip(hloc)
                    while pending:   # safety: never drop undrained units
                        pending.pop(0)()

                    # reciprocals emitted inline (DVE + pool only - no PE cost);
                    # the PE-visible units (broadcast, outproj) are paced into
                    # the next block after a delay covering this chain

                    def bcast_norm_unit(m, avq_all=avq_all, rbs=rbs):
                        # both h of the pair share one PSUM alloc and the same SEL
                        # stationary (one LDWEIGHTS): fewer sc-ring injections
                        def emit():
                            rcb = psA.tile([128, 1024], f32, tag="sc", bufs=3, name="rcb")
                            for hloc in range(2):
                                nc.tensor.matmul(rcb[0:64, 512 * hloc:512 * (hloc + 1)],
                                                 SELc[m], rbs[hloc][:], start=True, stop=True)
                            for hloc in range(2):
                                qb = 64 * hloc
                                nc.vector.tensor_tensor(avq_all[m][qb:qb + 64, :],
                                                        avq_all[m][qb:qb + 64, :],
                                                        rcb[0:64, 512 * hloc:512 * (hloc + 1)],
                                                        ALU.mult)
                        return emit

                    pending = ([bcast_norm_unit(m) for m in range(4)]
                               + [outproj_unit(avq_all, qt, j, npair)
                                  for j in range(4) for npair in range(2)])
                for emit in pending:
                    emit()

    nc.compile()
    return nc


_PERM64 = np.concatenate([np.arange(0, 64, 2), np.arange(1, 64, 2)])
# Q-tile m holds local heads (m, m+4) so each head's partition base (0/64) matches
# its KV group's base in KTrot (group g at rows 64g) - matmul requires equal bases.
_HEADS_ORDER = np.array([0, 4, 1, 5, 2, 6, 3, 7])
_BF16 = ml_dtypes.bfloat16


def _prep_core(c, x, freqs_cis, mask, wq, bq, wk, bk, wv, bv, wo, mode,
               has_bq, has_bk, has_bv):
    b, gp = divmod(c, 4)
    f = np.float32
    xT = np.ascontiguousarray(x[b].T.astype(_BF16))
    wq_c = wq[512 * gp:512 * (gp + 1)].reshape(8, 64, H)[_HEADS_ORDER][:, _PERM64, :].reshape(512, H)
    wqT = np.ascontiguousarray(wq_c.T.astype(_BF16))
    wk_c = wk[128 * gp:128 * (gp + 1)].reshape(2, 64, H)[:, _PERM64, :].reshape(128, H)
    wv_c = wv[128 * gp:128 * (gp + 1)]
    wkvT = np.ascontiguousarray(np.concatenate([wk_c, wv_c], 0).T.astype(_BF16))
    woR = wo[:, 512 * gp:512 * (gp + 1)].T.reshape(8, 64, H)[_HEADS_ORDER].reshape(512, H)
    woR = np.ascontiguousarray(woR.astype(_BF16))
    cosT = np.ascontiguousarray(freqs_cis[:, 0::2].T, dtype=f)   # (32, S)
    sinT = np.ascontiguousarray(freqs_cis[:, 1::2].T, dtype=f)
    COS = np.tile(cosT, (4, 1)).astype(_BF16)
    # partition blocks pre-swapped pairwise: block at base sb holds the sign-applied
    # sin coefficients of destination block db = sb ^ 32 (see rope in _build)
    SIN = np.concatenate([sinT, -sinT, sinT, -sinT], 0).astype(_BF16)
    i = np.arange(128)
    L = (i[:, None] < i[None, :]).astype(_BF16)          # L[k, m] = 1 if k < m
    Z = (np.float32(-8e9) * np.eye(128, dtype=f)).astype(_BF16)
    I = np.eye(128, dtype=f).astype(_BF16)
    SEL = []
    for m4 in range(4):
        s = np.zeros((128, 64), dtype=_BF16)
        s[32 * m4, :] = 1
        SEL.append(s)
    CST = np.concatenate([L, Z, I] + SEL, axis=1)
    m = {"xT": xT, "wqT": wqT, "wkvT": wkvT, "woR": woR,
         "COSx": np.ascontiguousarray(COS), "SINx": np.ascontiguousarray(SIN),
         "CST": np.ascontiguousarray(CST)}
    if mode == "generic":
        m["maskT"] = np.ascontiguousarray(mask.T, dtype=f)
    if has_bq:
        bq_c = bq[512 * gp:512 * (gp + 1)].reshape(8, 64)[_HEADS_ORDER][:, _PERM64].reshape(512, 1)
        m["bq"] = np.ascontiguousarray(bq_c, dtype=f)
    if has_bk or has_bv:
        bk_c = bk[128 * gp:128 * (gp + 1)].reshape(2, 64)[:, _PERM64].reshape(128)
        bv_c = bv[128 * gp:128 * (gp + 1)]
        m["bkv"] = np.ascontiguousarray(np.concatenate([bk_c, bv_c]).reshape(256, 1), dtype=f)
    return m


def _detect_mode(mask):
    causal = np.where(np.tril(np.ones((S, S), dtype=bool)), np.float32(0.0), np.float32(-1e9))
    if np.array_equal(mask, causal):
        return "causal"
    if not np.any(mask):
        return "zeros"
    return "generic"


def _run(inputs, trace=False):
    from concourse import bass_utils
    x = np.asarray(inputs["x"], dtype=np.float32)
    freqs_cis = np.asarray(inputs["freqs_cis"], dtype=np.float32)
    mask = np.asarray(inputs["mask"], dtype=np.float32)
    wq = np.asarray(inputs["wq"], dtype=np.float32)
    bq = np.asarray(inputs["bq"], dtype=np.float32)
    wk = np.asarray(inputs["wk"], dtype=np.float32)
    bk = np.asarray(inputs["bk"], dtype=np.float32)
    wv = np.asarray(inputs["wv"], dtype=np.float32)
    bv = np.asarray(inputs["bv"], dtype=np.float32)
    wo = np.asarray(inputs["wo"], dtype=np.float32)
    bo = np.asarray(inputs["bo"], dtype=np.float32)

    mode = _detect_mode(mask)
    has_bq = bool(np.any(bq))
    has_bk = bool(np.any(bk))
    has_bv = bool(np.any(bv))
    key = (mode, has_bq, has_bk, has_bv)
    if key not in _CACHE:
        _CACHE[key] = _build(*key)
    nc = _CACHE[key]

    in_maps = [
        _prep_core(c, x, freqs_cis, mask, wq, bq, wk, bk, wv, bv, wo, mode,
                   has_bq, has_bk, has_bv)
        for c in range(8)
    ]
    res = bass_utils.run_bass_kernel_spmd(nc, in_maps, core_ids=list(range(8)), trace=trace)
    partials = np.stack([res.results[c]["out"].astype(np.float32) for c in range(8)], 0)
    out = partials.reshape(2, 4, S, H).sum(axis=1) + bo[None, None, :]
    return out.astype(np.float32), res


def kernel(**inputs):
    out, _ = _run(inputs, trace=False)
    return out
